# revision 40
# baseline (speedup 1.0000x reference)
"""AttnBlock1D (BN + single-head 1x1-conv attention + residual) on 8 TRN2 cores.

Contract: kernel(**inputs) takes the FULL inputs from setup_inputs() and
returns the FULL output [4, 256, 4096] f32. ~143 us HW exec (baseline
bf16 kernel: ~237 us), norm-relative error ~6.8e-3 against the f32
reference (gate: 2e-2).

Sharding: 8 cores = 4 samples x 2 query-halves (data-parallel over B,
attention split over queries). Core i handles sample b = i // 2 and
queries [qh*2048, (qh+1)*2048), qh = i % 2. The host rolls x[b] along L
so each core's queries are the FIRST 2048 columns -- attention is
permutation-invariant over keys, so k/v built from the rolled layout give
identical softmax results; the SPMD program needs no per-core constants.

BatchNorm stats are computed locally on every core -- NO collective (any
cross-core sync puts the run-variable NEFF start skew across cores onto
the measured span). The three OTHER samples' sums come from the PE: a
host-transposed fp8 copy xT8 ([l, c] layout, keys on partitions) feeds
DoubleRow matmuls per 256-key pair that accumulate per-channel-block Gram
matrices (diagonal = sum x^2) and a ones-stationary row-sum whose
diagonal is sum x; one tensor_mul-with-identity + reduce_sum per block
extracts the diagonals (tensor_tensor_reduce would do it in one op but
faults on HW). The OWN sample runs through DVE bn_stats straight from the
fp8 x8 compute copy. This keeps the old ~45us DVE/ACT stats phase off
the critical path: startup is HBM-bound (all 8 cores stream ~6.5 MB
each), and the Gram matmuls double as PE warm-up. rsqrt(var+eps) is
exp(-0.5*ln(var+eps)) so every ACT function used (Copy/Ln/Exp) lives in
the single natural_log_exp_and_others table -- zero mid-kernel ACT table
loads (a dummy Ln at t~0 hoists the one 1.28us load under the DMA wait).

The BN affine folds into the projections: w8 = fp8(w * a[c]) per input
channel (ACT Copy with per-partition scale, fp8 out, emitted FIRST so the
k projection starts as soon as `a` exists); effective biases via N=1
matvecs on the raw bf16 weights overlap the k matmuls. The k bias drops
entirely (softmax shift-invariance); the v-path constant folds into the
output projection bias.

Everything heavy runs in fp8-e4m3 DoubleRow (contract 256 per
instruction, ~2.2x bf16 measured, ~209 ns per 512-col matmul): q/k/v
projections read x8 against w8; scores per key tile are one DoubleRow
matmul; exp on ACT (scale=1/16, bias=-3 folded in -- max scaled score ~8
overflows e4m3's 448 unshifted; softmax cancels the shift) writes fp8
probabilities keys-on-partitions; AV, the ones-matmul denominator
(which also broadcasts it across partitions for free), and the output
projection (host-quantized wp8) all contract DoubleRow from the same pT
tiles. PSUM->SBUF fp8 casts of k/q/v alternate DVE and ACT so neither
engine gates the first attention chunk. reciprocal_approx_fast + one
tensor_mul per channel-half normalizes out of PSUM; the residual adds a
bf16 copy of x (quantization ~4e-4 of the error budget, saves 2 MB of
HBM per core).

Queries run in 5 chunks (3x512 + 2x256) with double-buffered probability
tiles; chunk-0 scores are emitted before the v projection so ACT exp
starts early. During the projections the attention PSUM banks are idle,
so the k/q/v projection PSUMs rotate through FOUR banks (s-buf0, s-buf1,
den, o) -- with only two, each projection matmul stalls behind the
~0.6us PSUM->fp8 cast two tiles earlier and the first exp slips ~8us. In steady state the Tile scheduler keeps the PE at
back-to-back matmul rate through the scores+AV stream while ACT runs the
exp stream at ~1.07us/tile -- both engines are ~saturated. Measured
no-gos (each made it slower): explicit jp-level interleaving of
scores(n+1) with AV(n) (+50ns/matmul of PSUM contention), splitting each
exp tile's columns between ACT and a DVE Schraudolph bit-trick path, a
second DMA queue for xT8 (gpsimd SWDGE is slow; out-of-order chunk
arrival stalls the in-order Gram consumer), and issuing the residual DMA
late (it collides with the attention phase instead of the HBM-bound
stats phase), and 4x512 query chunks instead of 3x512+2x256 (+27us! the
smaller tail chunks' cheaper per-tile exp and lighter epilogues matter
far more than the 16 extra ACT instruction overheads).
"""

import os

import numpy as np
import ml_dtypes

import concourse.bass as bass
import concourse.mybir as mybir
import concourse.tile as tile
from concourse import bacc
from concourse import bass_utils

F32 = mybir.dt.float32
BF16 = mybir.dt.bfloat16
F8 = mybir.dt.float8e4
DR = mybir.MatmulPerfMode.DoubleRow

N_CORES = 8
B, C, L = 4, 256, 4096
M = L // 2          # queries per core
EPS = 1e-5
SCALE = 1.0 / 16.0  # C ** -0.5
CSHIFT = 3.0        # exp bias: p = exp(s/16 - CSHIFT); cancels in softmax

NJT = L // 128      # 32 key tiles
NJP = NJT // 2      # 16 key-tile pairs (DoubleRow contracts 256 keys)
NPAIR = (B - 1) * NJP   # 48 stat pairs: the 3 OTHER samples
NSTCH = 6               # xT8 arrives in 6 chunks of 8 pairs
AF = mybir.ActivationFunctionType

LAST_EXEC_NS = None
_COMPILED = None


def _build():
    nc = bacc.Bacc("TRN2", target_bir_lowering=False, debug=False,
                   num_devices=N_CORES)

    x_d = nc.dram_tensor("x", [128, 2, L], BF16, kind="ExternalInput")
    xT8_d = nc.dram_tensor("xT8", [128, NPAIR * 512], F8, kind="ExternalInput")
    x8_d = nc.dram_tensor("x8", [128, 2, L], F8, kind="ExternalInput")
    # all four weight matrices packed: [c_p, ch, (q,k,v,p), o]
    wall_d = nc.dram_tensor("wall", [128, 2 * 4 * C], BF16,
                            kind="ExternalInput")
    wp8_d = nc.dram_tensor("wp8", [128, 2 * C], F8, kind="ExternalInput")
    id_d = nc.dram_tensor("id128", [128, 128], BF16, kind="ExternalInput")
    # bq, bpe, gamma, beta packed as columns: [c_p, ch, 4]
    vp_d = nc.dram_tensor("vpack", [128, 2 * 4], F32, kind="ExternalInput")
    out_d = nc.dram_tensor("out", [C, M], F32, kind="ExternalOutput")

    with tile.TileContext(nc) as tc:
        with (
            tc.tile_pool(name="big", bufs=1) as big,
            tc.tile_pool(name="pt", bufs=2) as ptp,
            tc.tile_pool(name="small", bufs=2) as sm,
            tc.tile_pool(name="eps", bufs=3) as epi,
            tc.tile_pool(name="ps_s", bufs=2, space="PSUM") as ps_s,
            tc.tile_pool(name="ps_acc", bufs=1, space="PSUM") as ps_acc,
            tc.tile_pool(name="ps_o", bufs=1, space="PSUM") as ps_o,
        ):
            ones8 = big.tile([128, 2, 128], F8, name="ones8")
            nc.vector.memset(ones8[:], 1.0)
            csh = big.tile([128, 1], F32, name="csh")
            nc.vector.memset(csh[:], -CSHIFT)
            # dummy Ln: hoists the 1.28us ACT table load to t~0, hidden
            # under the xT8 DMA wait (Ln/Exp/Copy share one table)
            lnwarm = big.tile([128, 1], F32, name="lnwarm")
            nc.scalar.activation(lnwarm[:], csh[:], AF.Ln)

            # ------- BN stats on the PE: Gram diagonals + row sums --------
            # xT8 chunk tiles [128, 8, 2, 256]: (p, jp_local, i, c) holds
            # x8[c, jp*256 + i*128 + p] summed over all 4 samples' length.
            # xT8 is on the startup critical path: its chunk DMAs issue
            # FIRST on the sync queue (in-order arrival matters; a second
            # queue measured slower, not faster).
            g_ps = [ps_acc.tile([128, 128], F32, tag=f"av{h}", name=f"g{h}")
                    for h in range(2)]
            m_ps = ps_acc.tile([128, C], F32, tag="den", name="m_ps")
            PPC = NPAIR // NSTCH     # pairs per chunk
            x8 = big.tile([128, 2, L], F8, name="x8")
            for t in range(NSTCH):
                if t == 2:
                    # x8 rides the sync queue mid-stream: it feeds the
                    # own-sample bn_stats on the otherwise-idle DVE
                    nc.sync.dma_start(x8[:], x8_d[:, :, :])
                xst = sm.tile([128, PPC, 2, 256], F8, tag="xst", bufs=2,
                              name=f"xst{t}")
                nc.sync.dma_start(
                    xst[:], xT8_d[:, t * PPC * 512:(t + 1) * PPC * 512])
                for jpl in range(PPC):
                    jp = t * PPC + jpl
                    first, last = jp == 0, jp == NPAIR - 1
                    for h in range(2):
                        nc.tensor.matmul(
                            g_ps[h][:],
                            xst[:, jpl, :, h * 128:(h + 1) * 128],
                            xst[:, jpl, :, h * 128:(h + 1) * 128],
                            start=first, stop=last, perf_mode=DR,
                        )
                    nc.tensor.matmul(
                        m_ps[:], ones8[:], xst[:, jpl, :, :],
                        start=first, stop=last, perf_mode=DR,
                    )

            # own-sample stats on DVE bn_stats (fp8 x8, f32 accumulation)
            s6 = [sm.tile([128, 8 * 6], F32, name=f"s6_{h}")
                  for h in range(2)]
            for h in range(2):
                for i in range(8):
                    nc.vector.bn_stats(
                        s6[h][:, i * 6:(i + 1) * 6],
                        x8[:, h, i * 512:(i + 1) * 512])

            # small/late inputs stream in behind the stats chunks on the
            # scalar queue (also hardware-DGE; ACT is idle this early)
            id_t = big.tile([128, 128], BF16, name="id_t")
            nc.scalar.dma_start(id_t[:], id_d[:, :])
            vp_t = big.tile([128, 2, 4], F32, name="vp_t")
            nc.scalar.dma_start(vp_t[:], vp_d[:, :])
            vecs = {nm: [vp_t[:, h, i:i + 1] for h in range(2)]
                    for i, nm in enumerate(("bq", "bpe", "gam", "bet"))}

            wall = big.tile([128, 2, 4, C], BF16, name="wall")
            nc.scalar.dma_start(wall[:], wall_d[:, :])
            w_t = {nm: [wall[:, ch, i, :] for ch in range(2)]
                   for i, nm in enumerate(("q", "k", "v", "p"))}
            wp8 = big.tile([128, 2, C], F8, name="wp8t")
            nc.scalar.dma_start(wp8[:], wp8_d[:, :])

            # bf16 x arrives late; only the epilogue residual reads it.
            # Issued on the SYNC queue right after the xT8 chunks so its
            # 2MB transfer starts only once the stats input is done,
            # instead of competing with xT8 for the shared HBM cap.
            # (The failed variant issued it from the ACT engine mid-
            # stream; the sync engine is idle here.)
            x3 = big.tile([128, 2, L], BF16, name="x3")
            nc.sync.dma_start(x3[:], x_d[:, :, :])
            x_t = [x3[:, h, :] for h in range(2)]

            # ------- extract diagonals, combine -> a (scale), d (shift) ---
            # The two channel-halves' chains are independent: emit them
            # step-interleaved so each step's second op hides the first's
            # cross-op semaphore latency.
            NT = B * L
            N1 = float(L)        # own-sample count per channel
            t_ = lambda nm: [sm.tile([128, 1], F32, name=f"{nm}{h}")
                             for h in range(2)]
            scr = [sm.tile([128, 128], F32, tag="scr", bufs=4,
                           name=f"scrg{h}") for h in range(2)]
            scrm = [sm.tile([128, 128], F32, tag="scr", bufs=4,
                            name=f"scrm{h}") for h in range(2)]
            sq, mn, tot, e2o, totq = t_("sq"), t_("mn"), t_("tot"), t_("e2o"), t_("totq")
            ngm, ge2p, nvar, lnv, rs = t_("ngm"), t_("ge2p"), t_("nvar"), t_("lnv"), t_("rs")
            a_t, d_t = t_("a"), t_("d")
            s2 = [sm.tile([128, 2], F32, name=f"s2_{h}") for h in range(2)]
            for h in range(2):
                nc.vector.tensor_mul(scr[h][:], g_ps[h][:], id_t[:])
            for h in range(2):
                nc.vector.tensor_mul(scrm[h][:],
                                     m_ps[:, h * 128:(h + 1) * 128], id_t[:])
            for h in range(2):
                nc.vector.reduce_sum(sq[h][:], scr[h][:],
                                     axis=mybir.AxisListType.X)
            for h in range(2):
                nc.vector.reduce_sum(mn[h][:], scrm[h][:],
                                     axis=mybir.AxisListType.X)
            for h in range(2):
                nc.vector.bn_aggr(s2[h][:], s6[h][:])
            for h in range(2):   # total sum x
                nc.vector.scalar_tensor_tensor(
                    out=tot[h][:], in0=s2[h][:, 0:1], scalar=N1,
                    in1=mn[h][:],
                    op0=mybir.AluOpType.mult, op1=mybir.AluOpType.add)
            for h in range(2):   # own E[x^2] = mean^2 + var (one fused STT)
                nc.vector.scalar_tensor_tensor(
                    out=e2o[h][:], in0=s2[h][:, 0:1],
                    scalar=s2[h][:, 0:1], in1=s2[h][:, 1:2],
                    op0=mybir.AluOpType.mult, op1=mybir.AluOpType.add)
            for h in range(2):   # total sum x^2
                nc.vector.scalar_tensor_tensor(
                    out=totq[h][:], in0=e2o[h][:], scalar=N1,
                    in1=sq[h][:],
                    op0=mybir.AluOpType.mult, op1=mybir.AluOpType.add)
            for h in range(2):   # ngm = -mean
                nc.vector.tensor_scalar_mul(ngm[h][:], tot[h][:], -1.0 / NT)
            for h in range(2):   # ge2p = E[x^2] + EPS
                nc.vector.tensor_scalar(
                    out=ge2p[h][:], in0=totq[h][:], scalar1=1.0 / NT,
                    scalar2=EPS,
                    op0=mybir.AluOpType.mult, op1=mybir.AluOpType.add)
            for h in range(2):   # nvar = mean^2 - E[x^2] - EPS = -(var+eps)
                nc.vector.scalar_tensor_tensor(
                    out=nvar[h][:], in0=ngm[h][:], scalar=ngm[h][:],
                    in1=ge2p[h][:],
                    op0=mybir.AluOpType.mult, op1=mybir.AluOpType.subtract)
            for h in range(2):   # ln(var+eps): the negate folds into scale
                nc.scalar.activation(lnv[h][:], nvar[h][:], AF.Ln,
                                     scale=-1.0)
            for h in range(2):   # rsqrt = exp(-0.5 ln): one ACT table total
                nc.scalar.activation(rs[h][:], lnv[h][:], AF.Exp,
                                     scale=-0.5)
            for h in range(2):
                nc.vector.tensor_mul(a_t[h][:], rs[h][:], vecs["gam"][h])
            for h in range(2):
                nc.vector.scalar_tensor_tensor(
                    out=d_t[h][:], in0=a_t[h][:], scalar=ngm[h][:],
                    in1=vecs["bet"][h],
                    op0=mybir.AluOpType.mult, op1=mybir.AluOpType.add)

            # ------- fold BN affine into weights + effective biases -------
            # w8 = fp8(w[c, o] * a[c]) FIRST (k-projection needs only w8k);
            # the b*_eff matvec chains follow and overlap the k matmuls.
            w8 = {}
            for nm in ("k", "q", "v"):
                w8[nm] = big.tile([128, 2, C], F8, name=f"w8{nm}")
                for ch in range(2):
                    nc.scalar.activation(
                        w8[nm][:, ch, :], w_t[nm][ch], AF.Copy,
                        scale=a_t[ch][:])

            d16 = [sm.tile([128, 1], BF16, name=f"d16_{h}") for h in range(2)]
            for h in range(2):
                nc.vector.tensor_copy(d16[h][:], d_t[h][:])

            def matvec(wtiles, rhs16, name):
                """out[o] = sum_c w[o, c] * rhs[c] as [2][128, 1] sbuf f32"""
                outs = []
                for oh in range(2):
                    ps = ps_s.tile([128, 1], F32, tag="s", name=f"mv_{name}{oh}")
                    for ch in range(2):
                        nc.tensor.matmul(
                            ps[:],
                            wtiles[ch][:, oh * 128:(oh + 1) * 128],
                            rhs16[ch][:],
                            start=(ch == 0), stop=(ch == 1),
                        )
                    o = sm.tile([128, 1], F32, name=f"mvo_{name}{oh}")
                    nc.vector.tensor_copy(o[:], ps[:])
                    outs.append(o)
                return outs

            wqd = matvec(w_t["q"], d16, "q")
            wvd = matvec(w_t["v"], d16, "v")
            bq_e = []
            for oh in range(2):
                t = sm.tile([128, 1], F32, name=f"bqe{oh}")
                nc.vector.tensor_add(t[:], wqd[oh][:], vecs["bq"][oh])
                bq_e.append(t)
            # bpe_eff = bpe + wp @ (wv @ d)
            wvd16 = [sm.tile([128, 1], BF16, name=f"wvd16_{h}")
                     for h in range(2)]
            for h in range(2):
                nc.vector.tensor_copy(wvd16[h][:], wvd[h][:])
            wpwvd = matvec(w_t["p"], wvd16, "p")
            bp_e = []
            for oh in range(2):
                t = sm.tile([128, 1], F32, name=f"bpe_e{oh}")
                nc.vector.tensor_add(t[:], wpwvd[oh][:], vecs["bpe"][oh])
                bp_e.append(t)

            # ---------------- projections (fp8 DoubleRow) -----------------
            # k first (gates chunk-0 scores), then q, then v behind the
            # first score matmuls.
            q8 = big.tile([128, 2, M], F8, name="q8")
            k8 = big.tile([128, 2, L], F8, name="k8")
            vT8 = big.tile([128, NJP, 2, 256], F8, name="vT8")

            # PSUM->fp8 casts alternate DVE / ACT so neither engine gates
            # the first attention chunk
            _cast_n = [0]

            def cast_out(dst, src):
                i = _cast_n[0]; _cast_n[0] += 1
                if i % 2 == 0:
                    nc.vector.tensor_copy(dst, src)
                else:
                    nc.scalar.activation(dst, src, AF.Copy)

            # during projections the attention PSUM banks are idle: rotate
            # k/q ps through 4 banks (s-buf0, s-buf1, den, o) so the two
            # cast streams run in parallel instead of gating the PE
            _pj = [0]

            def proj_ps(name):
                i = _pj[0]; _pj[0] += 1
                tag = ("s", "s", "den", "o")[i % 4]
                pool = ps_o if tag == "o" else (
                    ps_acc if tag == "den" else ps_s)
                return pool.tile([128, 512], F32, tag=tag, name=name)

            for oh in range(2):
                for it in range(L // 512):
                    ps = proj_ps("ps_k")
                    nc.tensor.matmul(
                        ps[:],
                        w8["k"][:, :, oh * 128:(oh + 1) * 128],
                        x8[:, :, it * 512:(it + 1) * 512],
                        start=True, stop=True, perf_mode=DR,
                    )
                    # softmax shift-invariance: k needs no bias
                    cast_out(k8[:, oh, it * 512:(it + 1) * 512], ps[:])

            for oh in range(2):
                for it in range(M // 512):
                    ps = proj_ps("ps_q")
                    nc.tensor.matmul(
                        ps[:],
                        w8["q"][:, :, oh * 128:(oh + 1) * 128],
                        x8[:, :, it * 512:(it + 1) * 512],
                        start=True, stop=True, perf_mode=DR,
                    )
                    nc.vector.tensor_scalar_add(
                        q8[:, oh, it * 512:(it + 1) * 512], ps[:],
                        bq_e[oh][:])

            # ---------------- attention, chunk by chunk ----------------
            chunks = [(0, 512), (512, 512), (1024, 512),
                      (1536, 256), (1792, 256)]

            def emit_scores(cn, i0, chw):
                pT = ptp.tile([128, NJT, 512], F8, tag="pT", name=f"pT{cn}")
                for jp in range(NJP):
                    ps3 = ps_s.tile([128, 2, 512], F32, tag="s",
                                    name="ps_sc")
                    for half in range(2):
                        jt = jp * 2 + half
                        nc.tensor.matmul(
                            ps3[:, half, 0:chw],
                            k8[:, :, jt * 128:(jt + 1) * 128],
                            q8[:, :, i0:i0 + chw],
                            start=True, stop=True, perf_mode=DR,
                        )
                    nc.scalar.activation(
                        pT[:, jp * 2:jp * 2 + 2, 0:chw], ps3[:, :, 0:chw],
                        AF.Exp, scale=SCALE, bias=csh[:])
                return pT

            def emit_av(cn, i0, chw, pT):
                ps_av = [ps_acc.tile([128, chw], F32, tag=f"av{ch}",
                                     name=f"av{ch}_{cn}") for ch in range(2)]
                ps_den = ps_acc.tile([128, chw], F32, tag="den",
                                     name=f"den{cn}")
                for jp in range(NJP):
                    pslice = pT[:, jp * 2:jp * 2 + 2, 0:chw]
                    for ch in range(2):
                        nc.tensor.matmul(
                            ps_av[ch][:],
                            vT8[:, jp, :, ch * 128:(ch + 1) * 128],
                            pslice,
                            start=(jp == 0), stop=(jp == NJP - 1),
                            perf_mode=DR,
                        )
                    nc.tensor.matmul(
                        ps_den[:], ones8[:], pslice,
                        start=(jp == 0), stop=(jp == NJP - 1),
                        perf_mode=DR,
                    )

                rec = epi.tile([128, chw], F32, tag="rec", name=f"rec{cn}")
                nc.vector.reciprocal_approx_fast(rec[:], ps_den[:])

                at3 = epi.tile([128, 2, 512], F8, tag="at", name=f"at{cn}")
                for ch in range(2):
                    nc.vector.tensor_mul(
                        at3[:, ch, 0:chw], ps_av[ch][:], rec[:])

                for oh in range(2):
                    ps = ps_o.tile([128, chw], F32, tag="o", name=f"po{oh}_{cn}")
                    nc.tensor.matmul(
                        ps[:],
                        wp8[:, :, oh * 128:(oh + 1) * 128],
                        at3[:, :, 0:chw],
                        start=True, stop=True, perf_mode=DR,
                    )
                    res = epi.tile([128, chw], F32, tag="res",
                                   name=f"res{oh}_{cn}")
                    nc.vector.scalar_tensor_tensor(
                        out=res[:], in0=ps[:], scalar=bp_e[oh][:],
                        in1=x_t[oh][:, i0:i0 + chw],
                        op0=mybir.AluOpType.add, op1=mybir.AluOpType.add,
                    )
                    nc.sync.dma_start(
                        out_d[oh * 128:(oh + 1) * 128, i0:i0 + chw], res[:])

            # chunk-0 scores before the v projection: ACT exp starts while
            # the PE fills v
            pT0 = emit_scores(0, chunks[0][0], chunks[0][1])

            for jp in range(NJP):
                # v PSUM rotates av0, av1, den, o (all idle until chunk-0
                # AV, which needs vT8 complete anyway)
                vtag = ("av0", "av1", "den", "o")[jp % 4]
                vpool = ps_o if vtag == "o" else ps_acc
                psv = vpool.tile([128, 2, 256], F32, tag=vtag,
                                 name="ps_v")
                for half in range(2):
                    lt = jp * 2 + half
                    nc.tensor.matmul(
                        psv[:, half, :],
                        x8[:, :, lt * 128:(lt + 1) * 128],
                        w8["v"][:],
                        start=True, stop=True, perf_mode=DR,
                    )
                cast_out(vT8[:, jp, :, :], psv[:])

            prev = (0, chunks[0][0], chunks[0][1], pT0)
            for cn in range(1, len(chunks)):
                i0, chw = chunks[cn]
                pT = emit_scores(cn, i0, chw)
                emit_av(*prev)
                prev = (cn, i0, chw, pT)
            emit_av(*prev)

    nc.compile()
    return nc


_XT8_CACHE = None


def kernel(x, gamma, beta, wq, bq, wk, bk, wv, bv, wp, bp):
    global _COMPILED, LAST_EXEC_NS, _XT8_CACHE
    x = np.asarray(x, np.float32)
    if _COMPILED is None:
        _COMPILED = _build()
    nc = _COMPILED

    wp32 = np.asarray(wp, np.float32)
    wpT = np.ascontiguousarray(wp32.T)
    # wp8[c_p, ch*C + o] = fp8(wpT[ch*128 + c_p, o])
    wp8 = np.ascontiguousarray(
        wpT.reshape(2, 128, C).transpose(1, 0, 2).reshape(128, 2 * C)
    ).astype(ml_dtypes.float8_e4m3)

    # wall[c_p, ch, i, o] = wT_i[ch*128 + c_p, o], i in (q, k, v, p)
    wTs = [np.asarray(w, np.float32).T for w in (wq, wk, wv, wp)]
    wall = np.ascontiguousarray(
        np.stack(wTs, axis=0).reshape(4, 2, 128, C)
        .transpose(2, 1, 0, 3).reshape(128, 2 * 4 * C)
    ).astype(ml_dtypes.bfloat16)

    bpe = (np.asarray(bp, np.float32) + wp32 @ np.asarray(bv, np.float32))
    # vpack[c_p, ch, (bq, bpe, gamma, beta)]
    vpack = np.ascontiguousarray(
        np.stack([np.asarray(bq, np.float32), bpe,
                  np.asarray(gamma, np.float32),
                  np.asarray(beta, np.float32)], axis=1)
        .reshape(2, 128, 4).transpose(1, 0, 2).reshape(128, 8))

    common = {
        "wall": wall,
        "wp8": wp8,
        "id128": np.eye(128, dtype=ml_dtypes.bfloat16),
        "vpack": vpack,
    }

    x8 = [np.ascontiguousarray(x[b]).astype(ml_dtypes.float8_e4m3)
          for b in range(B)]
    # xT8[p, ((s*16 + jp)*2 + i)*256 + c] = x8[s][c, jp*256 + i*128 + p],
    # s over the 3 OTHER samples (own-sample stats run on-device bn_stats)
    xt_tiles = [x8[b].T.reshape(16, 2, 128, 256).transpose(2, 0, 1, 3)
                .reshape(128, 16 * 512) for b in range(B)]
    xT8s = [np.ascontiguousarray(np.concatenate(
                [xt_tiles[s] for s in range(B) if s != b], axis=1))
            for b in range(B)]

    in_maps = []
    for core in range(N_CORES):
        b, qh = core // 2, core % 2
        xb = x[b]
        if qh:
            xb = np.ascontiguousarray(np.roll(xb, -M, axis=1))
        # device layout [c_p, ch, l]
        xb3 = np.ascontiguousarray(xb.reshape(2, 128, L).transpose(1, 0, 2))
        in_maps.append({"x": xb3.astype(ml_dtypes.bfloat16),
                        "x8": xb3.astype(ml_dtypes.float8_e4m3),
                        "xT8": xT8s[b], **common})

    trace = os.environ.get("BASS_KERNEL_TRACE", "") == "1"
    res = bass_utils.run_bass_kernel_spmd(
        nc, in_maps, core_ids=list(range(N_CORES)), trace=trace)
    LAST_EXEC_NS = res.exec_time_ns

    out = np.empty((B, C, L), np.float32)
    for core in range(N_CORES):
        b, qh = core // 2, core % 2
        out[b, :, qh * M:(qh + 1) * M] = res.results[core]["out"]
    return out
